# revision 1
# baseline (speedup 1.0000x reference)
"""Trainium2 Bass kernel for nn_EndpointRegressor (2x TransformerConv GNN +
AttentionalAggregation) distributed over 8 NeuronCores.

Sharding: edges partitioned by destination node range (6272 nodes/core);
each core owns its dst nodes exclusively, so segment softmax/scatter stats
need no cross-core reduction.  Per layer each core computes k|v for its own
nodes, the k|v table is AllGather-ed, and each core gathers k|v rows for its
edge shard with dma_gather.  The segment softmax uses exp without max
subtraction (alpha range is ~±0.09 for this model family) and folds the
denominator division to the node side: out = (sum ex*v)/(sum ex).
Scatter-adds are one-hot matmuls accumulated in PSUM per 128-node window.
"""
import contextlib
import math
import numpy as np

def _nullcm():
    return contextlib.nullcontext()

import concourse.bass as bass
import concourse.bacc as bacc
import concourse.mybir as mybir
import concourse.tile as tile
from concourse._compat import get_trn_type
from concourse.bass_utils import run_bass_kernel_spmd
from concourse.library_config import mlp

# ---- problem constants (fixed by the problem spec) ----
N, E, G = 50000, 500000, 32
H, D = 4, 40
HID = H * D            # 160
JK = 2 * HID           # 320
NCORES = 8
NSHARD = 6272          # 49*128 nodes per core
NPAD = NCORES * NSHARD # 50176
WIN = NSHARD // 128    # 49
SPLIT = NPAD // 2      # 25088: low/high kv-table split (int16 gather indices)
GS = 1024              # slots per dma_gather instruction
INVSQD = 1.0 / math.sqrt(float(D))

f32 = mybir.dt.float32
i16 = mybir.dt.int16


def _wrap16(ix):
    """[n] int16 -> [128, n//16] dma_gather index layout (16-wrap, x8 replicate)."""
    return np.tile(ix.reshape(-1, 16).T, (8, 1))


def _preprocess(x, edge_index, edge_attr, batch):
    """Sort edges by dst, shard by dst range, split each window's edges into
    low/high src groups, pad to uniform chunk counts. Returns per-core input
    arrays + the adaptive chunk capacities (C_L, C_H)."""
    src = np.asarray(edge_index[0], dtype=np.int64)
    dst = np.asarray(edge_index[1], dtype=np.int64)
    ea = np.asarray(edge_attr, dtype=np.float32)
    order = np.argsort(dst, kind="stable")
    src, dst, ea = src[order], dst[order], ea[order]

    core = dst // NSHARD
    win = (dst % NSHARD) // 128
    low = src < SPLIT

    # bucket edge indices per (core, window, low/high)
    buckets = {}
    for r in range(NCORES):
        m_r = core == r
        for w in range(WIN):
            m = m_r & (win == w)
            idx = np.nonzero(m)[0]
            lo = idx[low[idx]]
            hi = idx[~low[idx]]
            buckets[(r, w)] = (lo, hi)

    C_L = max(1, max((len(b[0]) + 127) // 128 for b in buckets.values()))
    C_H = max(1, max((len(b[1]) + 127) // 128 for b in buckets.values()))
    NCH = C_L + C_H
    NGL = (WIN * C_L * 128 + GS - 1) // GS
    NGH = (WIN * C_H * 128 + GS - 1) // GS

    per_core = []
    for r in range(NCORES):
        Lslots = np.zeros(NGL * GS, np.int64)      # gather idx (low table)
        Lvalid = np.zeros(NGL * GS, bool)
        Hslots = np.zeros(NGH * GS, np.int64)
        Hvalid = np.zeros(NGH * GS, bool)
        eaT = np.zeros((WIN, 5, NCH * 128), np.float32)
        eaT[:, 4, :] = 1.0                         # bias row for the e-matmul
        dstrel = np.full((WIN, 128, NCH), -1.0, np.float32)
        for w in range(WIN):
            lo, hi = buckets[(r, w)]
            for (idx_e, slots, valid, base_c, j0, table_off) in (
                (lo, Lslots, Lvalid, w * C_L, 0, 0),
                (hi, Hslots, Hvalid, w * C_H, C_L, SPLIT),
            ):
                n = len(idx_e)
                s0 = base_c * 128
                slots[s0 : s0 + n] = src[idx_e] - table_off
                valid[s0 : s0 + n] = True
                # pipeline slot (w, j0 + k//128, k%128)
                kk = np.arange(n)
                jj = j0 + kk // 128
                pp = kk % 128
                eaT[w, :4, :][:, jj * 128 + pp] = ea[idx_e].T
                dstrel[w, pp, jj] = (dst[idx_e] % 128).astype(np.float32)
        # pad slots keep idx=0: every gather slot must be WRITTEN on hw
        # (unwritten SBUF can hold NaN garbage that poisons 0*NaN in the
        # scatter matmul), so no -1 skip sentinels.
        # own-node arrays
        n0 = r * NSHARD
        xT = np.zeros((5, NSHARD), np.float32)
        batchc = np.full((WIN, 128, 1), -1.0, np.float32)
        n_real = max(0, min(NSHARD, N - n0))
        if n_real > 0:
            xT[:, :n_real] = np.asarray(x[n0 : n0 + n_real], np.float32).T
            bc = np.asarray(batch[n0 : n0 + n_real], np.float32).reshape(-1, 1)
            batchc.reshape(NSHARD, 1)[:n_real] = bc
        per_core.append(
            dict(
                xT=xT,
                idxL=np.ascontiguousarray(_wrap16(Lslots.astype(np.int16))),
                idxH=np.ascontiguousarray(_wrap16(Hslots.astype(np.int16))),
                eaT=eaT,
                dstrel=dstrel,
                batchc=batchc,
            )
        )
    return per_core, C_L, C_H


def _weights(inp):
    """Host-side weight packing (bias folding, concat layouts)."""
    w = {}
    b_in = inp["b_in"].astype(np.float64)
    w["iota128"] = np.broadcast_to(np.arange(128, dtype=np.float32), (128, 128)).copy()
    w["iota32"] = np.broadcast_to(np.arange(32, dtype=np.float32), (128, 32)).copy()
    w["ident"] = np.eye(128, dtype=np.float32)
    Wg1 = inp["Wg1"].astype(np.float32)
    w["wg1_h1"] = np.concatenate([Wg1[:HID], inp["bg1"].astype(np.float32)[None, :]], 0)   # [161,160]
    w["wg1_h2"] = np.concatenate([Wg1[HID:], np.zeros((1, HID), np.float32)], 0)           # [161,160]
    w["wg2rep"] = np.broadcast_to(inp["Wg2"].astype(np.float32)[:, 0], (128, HID)).copy()
    w["wh1"] = np.concatenate([inp["Wh1"].astype(np.float32), inp["bh1"].astype(np.float32)[None, :]], 0)  # [321,320]
    w["wh2"] = np.concatenate([inp["Wh2"].astype(np.float32), inp["bh2"].astype(np.float32)[None, :]], 0)  # [321,6]
    w["win"] = inp["W_in"].astype(np.float32)  # [5,160]
    w["bg2rep"] = np.full((128, 1), float(np.asarray(inp["bg2"]).reshape(-1)[0]), np.float32)
    for layer in range(2):
        Wq, Wk, Wv = (inp[k][layer].astype(np.float64) for k in ("Wq", "Wk", "Wv"))
        bq, bk, bv = (inp[k][layer].astype(np.float64) for k in ("bq", "bk", "bv"))
        Wskip, bskip = inp["Wskip"][layer].astype(np.float64), inp["bskip"][layer].astype(np.float64)
        Wbeta = inp["Wbeta"][layer].astype(np.float64)
        We, be = inp["We"][layer].astype(np.float64), inp["be"][layer].astype(np.float64)
        if layer == 0:
            bq, bk, bv, bskip = bq + b_in @ Wq, bk + b_in @ Wk, bv + b_in @ Wv, bskip + b_in @ Wskip
        P = (Wbeta[:HID, 0] + Wbeta[2 * HID :, 0])
        Q = (Wbeta[HID : 2 * HID, 0] - Wbeta[2 * HID :, 0])
        w[f"wkv{layer}"] = np.concatenate(
            [np.concatenate([Wk, Wv], 1), np.concatenate([bk, bv])[None, :]], 0
        ).astype(np.float32)                                           # [161,320]
        w[f"wq{layer}"] = np.concatenate([Wq, bq[None, :]], 0).astype(np.float32)  # [161,160]
        rb = np.concatenate([Wskip, (Wskip @ Q)[:, None]], 1)          # [160,161]
        rb_b = np.concatenate([bskip, [bskip @ Q]])[None, :]           # [1,161]
        w[f"wrb{layer}"] = np.concatenate([rb, rb_b], 0).astype(np.float32)        # [161,161]
        w[f"prep{layer}"] = np.broadcast_to(P.astype(np.float32), (128, HID)).copy()
        w[f"wekv{layer}"] = np.concatenate(
            [np.concatenate([We, We], 1), np.concatenate([be, be])[None, :]], 0
        ).astype(np.float32)                                           # [5,320]
    return w


def _build(C_L, C_H, phases="full", winlim=None, skips=()):
    skips = set(skips)
    NCH = C_L + C_H
    NGL = (WIN * C_L * 128 + GS - 1) // GS
    NGH = (WIN * C_H * 128 + GS - 1) // GS

    nc = bacc.Bacc(get_trn_type() or "TRN2", target_bir_lowering=False)

    # ---- dram I/O ----
    d = {}
    d["xT"] = nc.dram_tensor("xT", [5, NSHARD], f32, kind="ExternalInput")
    d["idxL"] = nc.dram_tensor("idxL", [128, NGL * GS // 16], i16, kind="ExternalInput")
    d["idxH"] = nc.dram_tensor("idxH", [128, NGH * GS // 16], i16, kind="ExternalInput")
    d["eaT"] = nc.dram_tensor("eaT", [WIN, 5, NCH * 128], f32, kind="ExternalInput")
    d["dstrel"] = nc.dram_tensor("dstrel", [WIN, 128, NCH], f32, kind="ExternalInput")
    d["batchc"] = nc.dram_tensor("batchc", [WIN, 128, 1], f32, kind="ExternalInput")
    wshapes = dict(
        iota128=[128, 128], iota32=[128, 32], ident=[128, 128],
        wg1_h1=[161, HID], wg1_h2=[161, HID], wg2rep=[128, HID],
        wh1=[321, JK], wh2=[321, 6], win=[5, HID], bg2rep=[128, 1],
    )
    for layer in range(2):
        wshapes[f"wkv{layer}"] = [161, JK]
        wshapes[f"wq{layer}"] = [161, HID]
        wshapes[f"wrb{layer}"] = [161, 161]
        wshapes[f"prep{layer}"] = [128, HID]
        wshapes[f"wekv{layer}"] = [5, JK]
    for k, shp in wshapes.items():
        d[k] = nc.dram_tensor(k, shp, f32, kind="ExternalInput")
    out_d = nc.dram_tensor("out", [32, 6], f32, kind="ExternalOutput")
    dbg_d = nc.dram_tensor("dbg", [128, JK], f32, kind="ExternalOutput")

    hT = [nc.dram_tensor(f"hT{i}", [HID, NSHARD], f32) for i in range(3)]
    h_nm = [None, nc.dram_tensor("h_nm1", [NSHARD, HID], f32),
            nc.dram_tensor("h_nm2", [NSHARD, HID], f32)]
    kv_own = [nc.dram_tensor(f"kv_own{l}", [NSHARD, JK], f32) for l in range(2)]
    kv_full = [nc.dram_tensor(f"kv_full{l}", [NPAD, JK], f32, addr_space="Shared")
               for l in range(2)]
    pool_in = nc.dram_tensor("pool_in", [32, JK + 1], f32)
    pool_out = nc.dram_tensor("pool_out", [32, JK + 1], f32, addr_space="Shared")
    rg = [list(range(NCORES))]

    with tile.TileContext(nc) as tc:
        with (
            tc.tile_pool(name="const", bufs=1) as cst,
            tc.tile_pool(name="sb", bufs=2) as sb,
            tc.tile_pool(name="gath", bufs=3) as gath,
            tc.tile_pool(name="ps", bufs=2, space="PSUM") as ps,
        ):
            nc.gpsimd.load_library(mlp)
            regGS = nc.gpsimd.to_reg(GS)

            # ---- persistent constants ----
            C = {}
            def _load_const(key, part, cols, row0=0):
                t = cst.tile([part, cols], f32, name=f"c_{key}_{row0}")
                nc.sync.dma_start(out=t[:], in_=d[key][row0 : row0 + part, :])
                return t
            for layer in range(2):
                C[f"wkv{layer}a"] = _load_const(f"wkv{layer}", 128, JK)
                C[f"wkv{layer}b"] = _load_const(f"wkv{layer}", 32, JK, 128)
                C[f"wkv{layer}c"] = _load_const(f"wkv{layer}", 1, JK, 160)
                C[f"wq{layer}a"] = _load_const(f"wq{layer}", 128, HID)
                C[f"wq{layer}b"] = _load_const(f"wq{layer}", 32, HID, 128)
                C[f"wq{layer}c"] = _load_const(f"wq{layer}", 1, HID, 160)
                C[f"wrb{layer}a"] = _load_const(f"wrb{layer}", 128, 161)
                C[f"wrb{layer}b"] = _load_const(f"wrb{layer}", 32, 161, 128)
                C[f"wrb{layer}c"] = _load_const(f"wrb{layer}", 1, 161, 160)
                C[f"prep{layer}"] = _load_const(f"prep{layer}", 128, HID)
                C[f"wekv{layer}"] = _load_const(f"wekv{layer}", 5, JK)
            C["iota128"] = _load_const("iota128", 128, 128)
            C["iota32"] = _load_const("iota32", 128, 32)
            C["ident"] = _load_const("ident", 128, 128)
            C["wg2rep"] = _load_const("wg2rep", 128, HID)
            for key in ("wg1_h1", "wg1_h2"):
                C[key + "a"] = _load_const(key, 128, HID)
                C[key + "b"] = _load_const(key, 32, HID, 128)
                C[key + "c"] = _load_const(key, 1, HID, 160)
            C["bg2rep"] = _load_const("bg2rep", 128, 1)
            C["wh1a"] = _load_const("wh1", 128, JK)
            C["wh1b"] = _load_const("wh1", 128, JK, 128)
            C["wh1c"] = _load_const("wh1", 64, JK, 256)
            C["wh1d"] = _load_const("wh1", 1, JK, 320)
            C["wh2a"] = _load_const("wh2", 128, 6)
            C["wh2b"] = _load_const("wh2", 128, 6, 128)
            C["wh2c"] = _load_const("wh2", 64, 6, 256)
            C["wh2d"] = _load_const("wh2", 1, 6, 320)
            C["win"] = _load_const("win", 5, HID)

            idxLt = cst.tile([128, NGL * GS // 16], i16, name="idxLt")
            nc.sync.dma_start(out=idxLt[:], in_=d["idxL"][:])
            idxHt = cst.tile([128, NGH * GS // 16], i16, name="idxHt")
            nc.sync.dma_start(out=idxHt[:], in_=d["idxH"][:])

            # ---- phase 0: h0T = (x @ W_in)^T, own nodes ----
            with nc.named_scope("p0"):
                NT0 = (NSHARD + 511) // 512
                for t in range(NT0):
                    c0, cn = t * 512, min(512, NSHARD - t * 512)
                    xts = sb.tile([5, cn], f32, tag="xts")
                    nc.sync.dma_start(out=xts[:], in_=d["xT"][:, c0 : c0 + cn])
                    for (r0, m) in ((0, 128), (128, 32)):
                        ph = ps.tile([m, cn], f32, tag="kve", bufs=3)
                        nc.tensor.matmul(ph[:], C["win"][:, r0 : r0 + m], xts[:],
                                         start=True, stop=True)
                        hsb = sb.tile([m, cn], f32, tag="hsb")
                        nc.vector.tensor_copy(out=hsb[:], in_=ph[:])
                        nc.sync.dma_start(out=hT[0][r0 : r0 + m, c0 : c0 + cn], in_=hsb[:])

            ones1 = cst.tile([1, 128], f32, name="ones1")
            nc.gpsimd.memset(ones1[:], 1.0)

            # ==== two layers ====
            nlayers = 0 if phases == "p0" else (1 if phases in ("kv", "edge0") else 2)
            for layer in range(nlayers):
                hsrc = hT[layer]
                # ---- kv GEMM own nodes -> kv_own ----
                with nc.named_scope(f"kv{layer}"):
                    for t in range(WIN):
                        csl = slice(t * 128, (t + 1) * 128)
                        hta = sb.tile([128, 128], f32, tag="hta", bufs=3)
                        nc.sync.dma_start(out=hta[:], in_=hsrc[0:128, csl])
                        htb = sb.tile([32, 128], f32, tag="htb", bufs=3)
                        nc.sync.dma_start(out=htb[:], in_=hsrc[128:160, csl])
                        pkv = ps.tile([128, JK], f32, tag="kve", bufs=3)
                        nc.tensor.matmul(pkv[:], hta[:], C[f"wkv{layer}a"][:], start=True, stop=False)
                        nc.tensor.matmul(pkv[:], htb[:], C[f"wkv{layer}b"][:], start=False, stop=False)
                        nc.tensor.matmul(pkv[:], ones1[:, :128], C[f"wkv{layer}c"][:], start=False, stop=True)
                        kvsb = sb.tile([128, JK], f32, tag="kvsb")
                        nc.vector.tensor_copy(out=kvsb[:], in_=pkv[:])
                        nc.sync.dma_start(out=kv_own[layer][csl, :], in_=kvsb[:])
                with nc.named_scope(f"ag{layer}"):
                    nc.gpsimd.collective_compute(
                        "AllGather", mybir.AluOpType.bypass, replica_groups=rg,
                        ins=[kv_own[layer][:]], outs=[kv_full[layer][:]])
                if layer == 0:
                    dbgt = sb.tile([128, JK], f32, tag="dbgt")
                    nc.sync.dma_start(out=dbgt[:], in_=kv_full[0][13000:13128, :])
                    nc.sync.dma_start(out=dbg_d[:], in_=dbgt[:])

                # ---- edge phase ----
                if phases == "kv":
                    break
                with nc.named_scope(f"edge{layer}"):
                    cur = {"L": -1, "H": -1}
                    cur_tile = {"L": None, "H": None}

                    def _gather(region, gt):
                        if cur[region] == gt:
                            return cur_tile[region]
                        idxt, base, ng = (
                            (idxLt, 0, NGL) if region == "L" else (idxHt, SPLIT, NGH)
                        )
                        gtile = gath.tile([128, GS // 128, JK], f32, tag="g" + region)
                        nc.gpsimd.dma_gather(
                            gtile[:],
                            kv_full[layer][base : base + SPLIT, :],
                            idxt[:, gt * (GS // 16) : (gt + 1) * (GS // 16)],
                            num_idxs=GS, num_idxs_reg=regGS, elem_size=JK)
                        cur[region] = gt
                        cur_tile[region] = gtile
                        return gtile

                    for w in range(WIN if winlim is None else winlim):
                        wsl = slice(w * 128, (w + 1) * 128)
                        eaw = sb.tile([5, NCH * 128], f32, tag="eaw", bufs=3)
                        nc.sync.dma_start(out=eaw[:], in_=d["eaT"][w])
                        drw = sb.tile([128, NCH], f32, tag="drw", bufs=3)
                        nc.sync.dma_start(out=drw[:], in_=d["dstrel"][w])
                        hta = sb.tile([128, 128], f32, tag="hta", bufs=3)
                        nc.sync.dma_start(out=hta[:], in_=hsrc[0:128, wsl])
                        htb = sb.tile([32, 128], f32, tag="htb", bufs=3)
                        nc.sync.dma_start(out=htb[:], in_=hsrc[128:160, wsl])
                        # q for this window
                        pq = ps.tile([128, HID], f32, tag="qrb", bufs=1)
                        nc.tensor.matmul(pq[:], hta[:], C[f"wq{layer}a"][:], start=True, stop=False)
                        nc.tensor.matmul(pq[:], htb[:], C[f"wq{layer}b"][:], start=False, stop=False)
                        nc.tensor.matmul(pq[:], ones1[:, :128], C[f"wq{layer}c"][:], start=False, stop=True)
                        qw = sb.tile([128, HID], f32, tag="qw", bufs=3)
                        nc.vector.tensor_copy(out=qw[:], in_=pq[:])
                        # r / beta-partial for this window
                        prb = ps.tile([128, 161], f32, tag="qrb", bufs=1)
                        nc.tensor.matmul(prb[:], hta[:], C[f"wrb{layer}a"][:], start=True, stop=False)
                        nc.tensor.matmul(prb[:], htb[:], C[f"wrb{layer}b"][:], start=False, stop=False)
                        nc.tensor.matmul(prb[:], ones1[:, :128], C[f"wrb{layer}c"][:], start=False, stop=True)
                        rsb = sb.tile([128, 161], f32, tag="rsb", bufs=3)
                        nc.vector.tensor_copy(out=rsb[:], in_=prb[:])

                        pacc = ps.tile([128, 164], f32, tag="acc")
                        for j in range(NCH):
                            if j < C_L:
                                cidx = w * C_L + j
                                gtile = _gather("L", cidx // (GS // 128))
                            else:
                                cidx = w * C_H + (j - C_L)
                                gtile = _gather("H", cidx // (GS // 128))
                            sub = cidx % (GS // 128)
                            kv_g = gtile[:, sub, :]
                            # e = ea @ We (+bias) in PSUM
                            pe = ps.tile([128, JK], f32, tag="kve", bufs=3)
                            nc.tensor.matmul(pe[:], eaw[:, j * 128 : (j + 1) * 128],
                                             C[f"wekv{layer}"][:], start=True, stop=True)
                            # kv_e = kv_g + e
                            kve = sb.tile([128, JK], f32, tag="kvesb", bufs=4)
                            nc.vector.tensor_tensor(out=kve[:], in0=pe[:], in1=kv_g,
                                                    op=mybir.AluOpType.add)
                            # S^T one-hot [edges, nodes]
                            st = sb.tile([128, 128], f32, tag="st", bufs=4)
                            nc.vector.tensor_tensor(
                                out=st[:], in0=drw[:, j : j + 1].to_broadcast([128, 128]),
                                in1=C["iota128"][:], op=mybir.AluOpType.is_equal)
                            # S = (S^T)^T via PE transpose
                            if "qg" not in skips:
                                pst = ps.tile([128, 128], f32, tag="tp")
                                nc.tensor.transpose(pst[:], st[:], C["ident"][:])
                                ssb = sb.tile([128, 128], f32, tag="ssb", bufs=4)
                                nc.vector.tensor_copy(out=ssb[:], in_=pst[:])
                                # q gathered to edges
                                pqg = ps.tile([128, HID], f32, tag="tp")
                                nc.tensor.matmul(pqg[:], ssb[:], qw[:], start=True, stop=True)
                                qsrc = pqg[:]
                            else:
                                qsrc = kve[:, :HID]
                            # alpha = sum_d q_g * k_e per head
                            tq = sb.tile([128, HID], f32, tag="tq", bufs=4)
                            nc.vector.tensor_tensor(out=tq[:], in0=qsrc, in1=kve[:, :HID],
                                                    op=mybir.AluOpType.mult)
                            al = sb.tile([128, H], f32, tag="al", bufs=4)
                            nc.vector.tensor_reduce(
                                out=al[:], in_=tq[:].rearrange("p (h dd) -> p h dd", h=H),
                                axis=mybir.AxisListType.X, op=mybir.AluOpType.add)
                            # w tile: [v_e * ex | ex]
                            wt = sb.tile([128, 164], f32, tag="wt", bufs=4)
                            if "exp" not in skips:
                                nc.scalar.activation(out=wt[:, 160:164], in_=al[:],
                                                     func=mybir.ActivationFunctionType.Exp,
                                                     scale=INVSQD)
                            else:
                                nc.vector.tensor_copy(out=wt[:, 160:164], in_=al[:])
                            nc.vector.tensor_tensor(
                                out=wt[:, :HID].rearrange("p (h dd) -> p h dd", h=H),
                                in0=kve[:, HID:].rearrange("p (h dd) -> p h dd", h=H),
                                in1=wt[:, 160:164].rearrange("p (h o) -> p h o", h=H).to_broadcast([128, H, D]),
                                op=mybir.AluOpType.mult)
                            # scatter: acc[nodes] += S^T.T @ [w | ex]
                            nc.tensor.matmul(pacc[:], st[:], wt[:],
                                             start=(j == 0), stop=(j == NCH - 1),
                                             skip_group_check=True)

                        # ---- window post: out = num/den, beta gate, h' ----
                        accsb = sb.tile([128, 164], f32, tag="accsb")
                        nc.vector.tensor_copy(out=accsb[:], in_=pacc[:])
                        dmax = sb.tile([128, H], f32, tag="dmax")
                        nc.vector.tensor_scalar_max(dmax[:], accsb[:, 160:164], 1e-30)
                        denr = sb.tile([128, H], f32, tag="denr")
                        nc.vector.reciprocal(out=denr[:], in_=dmax[:])
                        outn = sb.tile([128, HID], f32, tag="outn")
                        nc.vector.tensor_tensor(
                            out=outn[:].rearrange("p (h dd) -> p h dd", h=H),
                            in0=accsb[:, :HID].rearrange("p (h dd) -> p h dd", h=H),
                            in1=denr[:].rearrange("p (h o) -> p h o", h=H).to_broadcast([128, H, D]),
                            op=mybir.AluOpType.mult)
                        scr = sb.tile([128, HID], f32, tag="scr")
                        outP = sb.tile([128, 1], f32, tag="outP")
                        nc.vector.tensor_tensor(out=scr[:], in0=outn[:],
                            in1=C[f"prep{layer}"][:], op=mybir.AluOpType.mult)
                        nc.vector.tensor_reduce(out=outP[:],
                            in_=scr[:].rearrange("p (a b) -> p a b", a=1),
                            axis=mybir.AxisListType.XY, op=mybir.AluOpType.add)
                        beta = sb.tile([128, 1], f32, tag="beta")
                        if "sig" not in skips:
                            nc.scalar.activation(out=beta[:], in_=outP[:],
                                                 func=mybir.ActivationFunctionType.Sigmoid,
                                                 bias=rsb[:, 160:161], scale=1.0)
                        else:
                            nc.vector.tensor_copy(out=beta[:], in_=outP[:])
                        dvec = sb.tile([128, HID], f32, tag="dvec")
                        nc.vector.tensor_sub(dvec[:], rsb[:, :HID], outn[:])
                        hp = sb.tile([128, HID], f32, tag="hp")
                        if "stt" not in skips:
                            nc.vector.scalar_tensor_tensor(
                                out=hp[:], in0=dvec[:], scalar=beta[:, 0:1], in1=outn[:],
                                op0=mybir.AluOpType.mult, op1=mybir.AluOpType.add)
                        else:
                            nc.vector.tensor_scalar_mul(hp[:], dvec[:], beta[:, 0:1])
                            nc.vector.tensor_add(hp[:], hp[:], outn[:])
                        nc.sync.dma_start(out=h_nm[layer + 1][wsl, :], in_=hp[:])
                        # transpose h' into hT[layer+1]
                        if "trans" in skips:
                            continue
                        ptr1 = ps.tile([128, 128], f32, tag="tp")
                        nc.tensor.transpose(ptr1[:], hp[:, 0:128], C["ident"][:])
                        t1 = sb.tile([128, 128], f32, tag="t1")
                        nc.vector.tensor_copy(out=t1[:], in_=ptr1[:])
                        nc.sync.dma_start(out=hT[layer + 1][0:128, wsl], in_=t1[:])
                        ptr2 = ps.tile([32, 128], f32, tag="tp")
                        nc.tensor.transpose(ptr2[:], hp[:, 128:160], C["ident"][:])
                        t2 = sb.tile([32, 128], f32, tag="t2")
                        nc.vector.tensor_copy(out=t2[:], in_=ptr2[:])
                        nc.sync.dma_start(out=hT[layer + 1][128:160, wsl], in_=t2[:])

            if phases == "p0":
                dbgt = sb.tile([128, JK], f32, tag="dbgt")
                nc.gpsimd.memset(dbgt[:], 0.0)
                nc.sync.dma_start(out=dbgt[:, :160], in_=hT[0][0:128, 999:1159])
                nc.sync.dma_start(out=dbg_d[:], in_=dbgt[:])
            # ==== final phase: gate + graph pooling + head MLP ====
            if phases != "full":
                dummy = sb.tile([32, 6], f32, tag="osb")
                nc.gpsimd.memset(dummy[:], 0.0)
                nc.sync.dma_start(out=out_d[:], in_=dummy[:])
            if phases == "full":
              with nc.named_scope("final"):
                pgr = ps.tile([32, JK + 1], f32, tag="acc")
                for w in range(WIN):
                    wsl = slice(w * 128, (w + 1) * 128)
                    h1w = sb.tile([128, HID], f32, tag="h1w")
                    nc.sync.dma_start(out=h1w[:], in_=h_nm[1][wsl, :])
                    h2w = sb.tile([128, HID], f32, tag="h2w")
                    nc.sync.dma_start(out=h2w[:], in_=h_nm[2][wsl, :])
                    bcw = sb.tile([128, 1], f32, tag="bcw")
                    nc.sync.dma_start(out=bcw[:], in_=d["batchc"][w])
                    pg = ps.tile([128, HID], f32, tag="kve", bufs=3)
                    first = True
                    for (src_hT, wkey) in ((hT[1], "wg1_h1"), (hT[2], "wg1_h2")):
                        g_a = sb.tile([128, 128], f32, tag="hta", bufs=3)
                        nc.sync.dma_start(out=g_a[:], in_=src_hT[0:128, wsl])
                        g_b = sb.tile([32, 128], f32, tag="htb", bufs=3)
                        nc.sync.dma_start(out=g_b[:], in_=src_hT[128:160, wsl])
                        nc.tensor.matmul(pg[:], g_a[:], C[wkey + "a"][:], start=first, stop=False)
                        first = False
                        nc.tensor.matmul(pg[:], g_b[:], C[wkey + "b"][:], start=False, stop=False)
                    nc.tensor.matmul(pg[:], ones1[:, :128], C["wg1_h1c"][:], start=False, stop=True)
                    grelu = sb.tile([128, HID], f32, tag="grelu")
                    nc.scalar.activation(out=grelu[:], in_=pg[:],
                                         func=mybir.ActivationFunctionType.Relu)
                    scr2 = sb.tile([128, HID], f32, tag="scr")
                    gatec = sb.tile([128, 1], f32, tag="gatec")
                    nc.vector.tensor_tensor(out=scr2[:], in0=grelu[:],
                        in1=C["wg2rep"][:], op=mybir.AluOpType.mult)
                    nc.vector.tensor_reduce(out=gatec[:],
                        in_=scr2[:].rearrange("p (a b) -> p a b", a=1),
                        axis=mybir.AxisListType.XY, op=mybir.AluOpType.add)
                    ge = sb.tile([128, 1], f32, tag="ge")
                    nc.scalar.activation(out=ge[:], in_=gatec[:],
                                         func=mybir.ActivationFunctionType.Exp,
                                         bias=C["bg2rep"][:, 0:1])
                    sg = sb.tile([128, 32], f32, tag="sg")
                    nc.vector.tensor_tensor(out=sg[:], in0=bcw[:].to_broadcast([128, 32]),
                                            in1=C["iota32"][:], op=mybir.AluOpType.is_equal)
                    wg = sb.tile([128, JK + 1], f32, tag="wg")
                    nc.vector.tensor_scalar_mul(wg[:, 0:HID], h1w[:], ge[:, 0:1])
                    nc.vector.tensor_scalar_mul(wg[:, HID:JK], h2w[:], ge[:, 0:1])
                    nc.vector.tensor_copy(out=wg[:, JK : JK + 1], in_=ge[:])
                    nc.tensor.matmul(pgr[:], sg[:], wg[:], start=(w == 0),
                                     stop=(w == WIN - 1), skip_group_check=True)
                pg_sb = sb.tile([32, JK + 1], f32, tag="pg_sb")
                nc.vector.tensor_copy(out=pg_sb[:], in_=pgr[:])
                nc.sync.dma_start(out=pool_in[:], in_=pg_sb[:])
                nc.gpsimd.collective_compute(
                    "AllReduce", mybir.AluOpType.add, replica_groups=rg,
                    ins=[pool_in[:]], outs=[pool_out[:]])
                psb = sb.tile([32, JK + 1], f32, tag="psb")
                nc.sync.dma_start(out=psb[:], in_=pool_out[:])
                gden = sb.tile([32, 1], f32, tag="gden")
                nc.vector.tensor_scalar_max(gden[:], psb[:, JK : JK + 1], 1e-30)
                gdr = sb.tile([32, 1], f32, tag="gdr")
                nc.vector.reciprocal(out=gdr[:], in_=gden[:])
                pl = sb.tile([32, JK], f32, tag="pl")
                nc.vector.tensor_scalar_mul(pl[:], psb[:, 0:JK], gdr[:, 0:1])

                def _headmm(vin, wa, wb, wc, wd, nout, tagp):
                    """vin [32, 320] @ W[320, nout] + bias via PE transposes."""
                    pouts = ps.tile([32, nout], f32, tag=tagp, bufs=(3 if tagp == "kve" else 1))
                    for si, (c0, m) in enumerate(((0, 128), (128, 128), (256, 64))):
                        ptt = ps.tile([m, 32], f32, tag="tp")
                        nc.tensor.transpose(ptt[:], vin[:, c0 : c0 + m], C["ident"][0:32, 0:32])
                        tsb = sb.tile([m, 32], f32, tag="tsb")
                        nc.vector.tensor_copy(out=tsb[:], in_=ptt[:])
                        nc.tensor.matmul(pouts[:], tsb[:], (wa, wb, wc)[si][:m, :],
                                         start=(si == 0), stop=False, skip_group_check=True)
                    nc.tensor.matmul(pouts[:], ones1[:, :32], wd[:],
                                     start=False, stop=True, skip_group_check=True)
                    return pouts

                ph1 = _headmm(pl, C["wh1a"], C["wh1b"], C["wh1c"], C["wh1d"], JK, "qrb")
                vrel = sb.tile([32, JK], f32, tag="vrel")
                nc.scalar.activation(out=vrel[:], in_=ph1[:],
                                     func=mybir.ActivationFunctionType.Relu)
                ph2 = _headmm(vrel, C["wh2a"], C["wh2b"], C["wh2c"], C["wh2d"], 6, "kve")
                osb = sb.tile([32, 6], f32, tag="osb")
                nc.vector.tensor_copy(out=osb[:], in_=ph2[:])
                nc.sync.dma_start(out=out_d[:], in_=osb[:])

    nc.compile()
    return nc


_CACHE = {}
_LAST_RES = None


def kernel(**inputs):
    inputs = {k: np.asarray(v) for k, v in inputs.items()}
    per_core, C_L, C_H = _preprocess(
        inputs["x"], inputs["edge_index"], inputs["edge_attr"], inputs["batch"])
    w = _weights(inputs)
    import os as _os
    phases = _os.environ.get("KERNEL_PHASES", "full")
    winlim = _os.environ.get("KERNEL_WINLIM")
    winlim = int(winlim) if winlim else None
    skips = tuple(s for s in _os.environ.get("KERNEL_SKIP", "").split(",") if s)
    key = (C_L, C_H, phases, winlim, skips)
    if key not in _CACHE:
        _CACHE[key] = _build(C_L, C_H, phases, winlim, skips)
    nc = _CACHE[key]
    in_maps = []
    for r in range(NCORES):
        m = dict(w)
        m.update(per_core[r])
        in_maps.append(m)
    import os
    trace = bool(os.environ.get("KERNEL_TRACE"))
    if trace:
        try:
            import axon_prof
            axon_prof.install()
        except Exception:
            trace = False
    res = run_bass_kernel_spmd(nc, in_maps, core_ids=list(range(NCORES)), trace=trace)
    if trace and res.exec_time_ns is not None:
        print(f"HW exec time: {res.exec_time_ns} ns")
        if res.per_core_scope_times:
            for scope, cores in sorted(res.per_core_scope_times.items()):
                print(f"  scope {scope}: {cores}")
    global _LAST_RES
    _LAST_RES = res
    out = res.results[0]["out"]
    return out.reshape(G, 2, 3).astype(np.float32)



# revision 21
# speedup vs baseline: 2.6087x; 2.6087x over previous
"""Trainium2 Bass kernel for nn_EndpointRegressor (2x TransformerConv GNN +
AttentionalAggregation) distributed over 8 NeuronCores.  v2: bf16 datapath.

Sharding: edges partitioned by destination node range (6272 nodes/core);
each core owns its dst nodes exclusively, so segment softmax/scatter stats
need no cross-core reduction.  Per layer each core computes its nodes'
k|v table (384-col bf16 rows, biases + edge bias folded), AllGathers it,
and dma_gathers rows for its edge shard.  The per-edge projection
e = edge_attr @ We is never materialized: its alpha contribution comes via
a node-level C table (C[n,h,c] = q[n,h]·We[c,h], gathered to edges through
the one-hot st_T matmul together with q), and its value contribution via
scattered stats S[n,h,c] = sum_e ex*ea_c followed by a per-window rank-16
correction matmul S @ WeP.  Segment softmax uses exp without max
subtraction (alpha ~ ±0.1 for this model family); the denominator is the
c=4 (ones) column of S.  One-hot scatter/gather matrices are host-built
and streamed as bf16; all matmuls are bf16 (FWL fast-weight-load active),
accumulation stays in fp32 PSUM.
"""
import math
import numpy as np
import ml_dtypes

import concourse.bass as bass
import concourse.bacc as bacc
import concourse.mybir as mybir
import concourse.tile as tile
from concourse._compat import get_trn_type
from concourse.bass_utils import run_bass_kernel_spmd
from concourse.library_config import mlp

# ---- problem constants ----
N, E, G = 50000, 500000, 32
H, D = 4, 40
HID = H * D            # 160
NCORES = 8
NSHARD = 6272          # 49*128 nodes per core
NPAD = NCORES * NSHARD # 50176
WIN = NSHARD // 128    # 49
SPLIT = NPAD // 2      # 25088 (int16 gather indices => 2 tables)
NG = WIN               # one gather per window per stream (num_idxs <= 1024!)
INVSQD = 1.0 / math.sqrt(float(D))

KVROW = 384            # [k 160 | v 160 | pad 64]
QCROW = 192            # [q 160 | C 20 | pad 12]
QCW = QCROW + 161      # + [r 160 | -rQ 1] = 353

f32 = mybir.dt.float32
bf16 = mybir.dt.bfloat16
i16 = mybir.dt.int16
npbf = ml_dtypes.bfloat16

AF = mybir.ActivationFunctionType


def _wrap16(ix):
    """[n] int16 -> [128, n//16] dma_gather index layout (16-wrap, x8 replicate)."""
    return np.tile(ix.reshape(-1, 16).T, (8, 1))


def _preprocess(x, edge_index, edge_attr, batch):
    src = np.asarray(edge_index[0], dtype=np.int64)
    dst = np.asarray(edge_index[1], dtype=np.int64)
    ea = np.asarray(edge_attr, dtype=np.float32)
    order = np.argsort(dst, kind="stable")
    src, dst, ea = src[order], dst[order], ea[order]

    core = dst // NSHARD
    win = (dst % NSHARD) // 128
    low = src < SPLIT

    buckets = {}
    for r in range(NCORES):
        m_r = core == r
        for w in range(WIN):
            m = m_r & (win == w)
            idx = np.nonzero(m)[0]
            buckets[(r, w)] = (idx[low[idx]], idx[~low[idx]])

    C_L = max(1, max((len(b[0]) + 127) // 128 for b in buckets.values()))
    C_H = max(1, max((len(b[1]) + 127) // 128 for b in buckets.values()))
    NCH = C_L + C_H
    GS_L, GS_H = C_L * 128, C_H * 128

    per_core = []
    for r in range(NCORES):
        Lslots = np.zeros(NG * GS_L, np.int64)
        Hslots = np.zeros(NG * GS_H, np.int64)
        eaC = np.zeros((WIN, 128, NCH, 8), np.float32)
        stq = np.zeros((WIN, 128, NCH * 128), np.float32)  # st_T [node, (chunk, edge)]
        sts = np.zeros((WIN, 128, NCH * 128), np.float32)  # st [edge, (chunk, node)]
        for w in range(WIN):
            lo, hi = buckets[(r, w)]
            for (idx_e, slots, Cg, j0, table_off) in (
                (lo, Lslots, C_L, 0, 0),
                (hi, Hslots, C_H, C_L, SPLIT),
            ):
                n = len(idx_e)
                s0 = w * Cg * 128
                slots[s0:s0 + n] = src[idx_e] - table_off
                kk = np.arange(n)
                jj = j0 + kk // 128
                pp = kk % 128
                dr = (dst[idx_e] % 128).astype(np.int64)
                eaC[w, pp, jj, 0:4] = ea[idx_e]
                stq[w, dr, jj * 128 + pp] = 1.0
                sts[w, pp, jj * 128 + dr] = 1.0
        # own-node arrays
        n0 = r * NSHARD
        x6T = np.zeros((6, NSHARD), np.float32)
        x6T[5, :] = 1.0
        sgw = np.zeros((WIN, 128, 32), np.float32)
        n_real = max(0, min(NSHARD, N - n0))
        if n_real > 0:
            x6T[:5, :n_real] = np.asarray(x[n0:n0 + n_real], np.float32).T
            bc = np.asarray(batch[n0:n0 + n_real], np.int64)
            sgw.reshape(NSHARD, 32)[np.arange(n_real), bc] = 1.0
        per_core.append(
            dict(
                x6T=x6T.astype(npbf),
                idxL=np.ascontiguousarray(_wrap16(Lslots.astype(np.int16))),
                idxH=np.ascontiguousarray(_wrap16(Hslots.astype(np.int16))),
                eaC=np.ascontiguousarray(eaC).astype(npbf),
                stq=stq.astype(npbf),
                sts=sts.astype(npbf),
                sgw=sgw.astype(npbf),
            )
        )
    return per_core, C_L, C_H


def _weights(inp):
    """Host-side weight packing (f64 folds -> bf16)."""
    w = {}
    W_in = np.asarray(inp["W_in"], np.float64)
    b_in = np.asarray(inp["b_in"], np.float64)
    for l in range(2):
        Wq, bq = inp["Wq"][l].astype(np.float64), inp["bq"][l].astype(np.float64)
        Wk, bk = inp["Wk"][l].astype(np.float64), inp["bk"][l].astype(np.float64)
        Wv, bv = inp["Wv"][l].astype(np.float64), inp["bv"][l].astype(np.float64)
        We, be = inp["We"][l].astype(np.float64), inp["be"][l].astype(np.float64)
        Wskip, bskip = inp["Wskip"][l].astype(np.float64), inp["bskip"][l].astype(np.float64)
        Wbeta = inp["Wbeta"][l].astype(np.float64)
        P = Wbeta[:HID, 0] + Wbeta[2 * HID:, 0]
        Q = Wbeta[HID:2 * HID, 0] - Wbeta[2 * HID:, 0]
        # WeP [16, 160]: rows (h, c<4) -> We[c, h*D:(h+1)*D] in cols h*D..
        WeP = np.zeros((16, HID), np.float64)
        WeC = np.zeros((HID, 16), np.float64)
        for h in range(H):
            for c in range(4):
                WeP[h * 4 + c, h * D:(h + 1) * D] = We[c, h * D:(h + 1) * D]
                WeC[h * D:(h + 1) * D, h * 4 + c] = We[c, h * D:(h + 1) * D]
        if l == 0:
            Wq_e = W_in @ Wq; bq_e = b_in @ Wq + bq
            Wk_e = W_in @ Wk; bk_e = b_in @ Wk + bk + be
            Wv_e = W_in @ Wv; bv_e = b_in @ Wv + bv + be
            Ws_e = W_in @ Wskip; bs_e = b_in @ Wskip + bskip
        else:
            Wq_e, bq_e = Wq, bq
            Wk_e, bk_e = Wk, bk + be
            Wv_e, bv_e = Wv, bv + be
            Ws_e, bs_e = Wskip, bskip
        nin = Wq_e.shape[0]
        kv_slab = np.zeros((nin + 1, KVROW), np.float64)
        kv_slab[:nin, 0:160] = Wk_e
        kv_slab[nin, 0:160] = bk_e
        kv_slab[:nin, 160:320] = Wv_e
        kv_slab[nin, 160:320] = bv_e
        qc_slab = np.zeros((nin + 1, QCW), np.float64)
        qc_slab[:nin, 0:160] = Wq_e
        qc_slab[nin, 0:160] = bq_e
        qc_slab[:nin, 160:176] = Wq_e @ WeC
        qc_slab[nin, 160:176] = bq_e @ WeC
        qc_slab[:nin, QCROW:QCROW + 160] = Ws_e
        qc_slab[nin, QCROW:QCROW + 160] = bs_e
        qc_slab[:nin, QCROW + 160] = -(Ws_e @ Q)
        qc_slab[nin, QCROW + 160] = -(bs_e @ Q)
        if l == 0:
            w["kvslab0"] = kv_slab.astype(npbf)       # [6, 384]
            w["qcslab0"] = qc_slab.astype(npbf)       # [6, 353]
        else:
            w["kvslab1"] = kv_slab.astype(npbf)       # [161, 384]
            w["qcslab1"] = qc_slab.astype(npbf)       # [161, 353]
        w[f"wep{l}"] = WeP.astype(npbf)               # [16, 160]
        w[f"prep{l}"] = np.broadcast_to(P, (128, HID)).astype(npbf).copy()
    w["ident"] = np.eye(128).astype(npbf)
    Wg1 = np.asarray(inp["Wg1"], np.float64)
    w["wg1h1"] = np.concatenate([Wg1[:HID], np.asarray(inp["bg1"], np.float64)[None, :]], 0).astype(npbf)  # [161,160]
    w["wg1h2"] = np.concatenate([Wg1[HID:], np.zeros((1, HID))], 0).astype(npbf)
    w["wg2rep"] = np.broadcast_to(np.asarray(inp["Wg2"], np.float64)[:, 0], (128, HID)).astype(npbf).copy()
    w["bg2rep"] = np.full((128, 1), float(np.asarray(inp["bg2"]).reshape(-1)[0])).astype(npbf)
    w["wh1"] = np.concatenate([np.asarray(inp["Wh1"], np.float64),
                               np.asarray(inp["bh1"], np.float64)[None, :]], 0).astype(npbf)  # [321,320]
    w["wh2"] = np.concatenate([np.asarray(inp["Wh2"], np.float64),
                               np.asarray(inp["bh2"], np.float64)[None, :]], 0).astype(npbf)  # [321,6]
    return w


def _build(C_L, C_H):
    NCH = C_L + C_H
    GS_L, GS_H = C_L * 128, C_H * 128
    assert GS_L <= 1024 and GS_H <= 1024, 'dma_gather num_idxs must be <= 1024'

    nc = bacc.Bacc(get_trn_type() or "TRN2", target_bir_lowering=False)

    d = {}
    d["x6T"] = nc.dram_tensor("x6T", [6, NSHARD], bf16, kind="ExternalInput")
    d["idxL"] = nc.dram_tensor("idxL", [128, NG * GS_L // 16], i16, kind="ExternalInput")
    d["idxH"] = nc.dram_tensor("idxH", [128, NG * GS_H // 16], i16, kind="ExternalInput")
    d["eaC"] = nc.dram_tensor("eaC", [WIN, 128, NCH, 8], bf16, kind="ExternalInput")
    d["stq"] = nc.dram_tensor("stq", [WIN, 128, NCH * 128], bf16, kind="ExternalInput")
    d["sts"] = nc.dram_tensor("sts", [WIN, 128, NCH * 128], bf16, kind="ExternalInput")
    d["sgw"] = nc.dram_tensor("sgw", [WIN, 128, 32], bf16, kind="ExternalInput")
    wshapes = dict(
        kvslab0=[6, KVROW], qcslab0=[6, QCW],
        kvslab1=[161, KVROW], qcslab1=[161, QCW],
        wep0=[16, HID], wep1=[16, HID], prep0=[128, HID], prep1=[128, HID],
        ident=[128, 128], wg1h1=[161, HID], wg1h2=[161, HID],
        wg2rep=[128, HID], bg2rep=[128, 1], wh1=[321, 320], wh2=[321, 6],
    )
    for k, shp in wshapes.items():
        d[k] = nc.dram_tensor(k, shp, bf16, kind="ExternalInput")
    out_d = nc.dram_tensor("out", [32, 6], f32, kind="ExternalOutput")
    dbg_d = nc.dram_tensor("dbg", [128, KVROW], f32, kind="ExternalOutput")

    kv_own = [nc.dram_tensor(f"kv_own{l}", [NSHARD, KVROW], bf16) for l in range(2)]
    kv_full = [nc.dram_tensor(f"kv_full{l}", [NPAD, KVROW], bf16, addr_space="Shared")
               for l in range(2)]
    hT = [None, nc.dram_tensor("hT1", [HID, NSHARD], bf16),
          nc.dram_tensor("hT2", [HID, NSHARD], bf16)]
    h_nm = [None, nc.dram_tensor("h_nm1", [NSHARD, HID], bf16),
            nc.dram_tensor("h_nm2", [NSHARD, HID], bf16)]
    pool_in = nc.dram_tensor("pool_in", [32, 321], f32)
    pool_out = nc.dram_tensor("pool_out", [32, 321], f32, addr_space="Shared")
    rg = [list(range(NCORES))]

    with tile.TileContext(nc) as tc:
        with (
            tc.tile_pool(name="const", bufs=1) as cst,
            tc.tile_pool(name="sb", bufs=2) as sb,
            tc.tile_pool(name="gath", bufs=3) as gath,
            tc.tile_pool(name="ps", bufs=2, space="PSUM") as ps,
        ):
            nc.gpsimd.load_library(mlp)
            regGS_L = nc.gpsimd.to_reg(GS_L)
            regGS_H = nc.gpsimd.to_reg(GS_H)

            C = {}
            def _load_const(key, part, cols, row0=0, dt=bf16):
                t = cst.tile([part, cols], dt, name=f"c_{key}_{row0}")
                nc.sync.dma_start(out=t[:], in_=d[key][row0:row0 + part, :])
                return t
            C["kvslab0"] = _load_const("kvslab0", 6, KVROW)
            C["qcslab0"] = _load_const("qcslab0", 6, QCW)
            C["kvslab1a"] = _load_const("kvslab1", 128, KVROW)
            C["kvslab1b"] = _load_const("kvslab1", 32, KVROW, 128)
            C["kvslab1c"] = _load_const("kvslab1", 1, KVROW, 160)
            C["qcslab1a"] = _load_const("qcslab1", 128, QCW)
            C["qcslab1b"] = _load_const("qcslab1", 32, QCW, 128)
            C["qcslab1c"] = _load_const("qcslab1", 1, QCW, 160)
            for l in range(2):
                C[f"wep{l}"] = _load_const(f"wep{l}", 16, HID)
                C[f"prep{l}"] = _load_const(f"prep{l}", 128, HID)
            C["ident"] = _load_const("ident", 128, 128)
            for key in ("wg1h1", "wg1h2"):
                C[key + "a"] = _load_const(key, 128, HID)
                C[key + "b"] = _load_const(key, 32, HID, 128)
            C["wg1bias"] = _load_const("wg1h1", 1, HID, 160)
            C["wg2rep"] = _load_const("wg2rep", 128, HID)
            C["bg2rep"] = _load_const("bg2rep", 128, 1)
            C["wh1a"] = _load_const("wh1", 128, 320)
            C["wh1b"] = _load_const("wh1", 128, 320, 128)
            C["wh1c"] = _load_const("wh1", 64, 320, 256)
            C["wh1d"] = _load_const("wh1", 1, 320, 320)
            C["wh2a"] = _load_const("wh2", 128, 6)
            C["wh2b"] = _load_const("wh2", 128, 6, 128)
            C["wh2c"] = _load_const("wh2", 64, 6, 256)
            C["wh2d"] = _load_const("wh2", 1, 6, 320)

            idxLt = cst.tile([128, NG * GS_L // 16], i16, name="idxLt")
            nc.sync.dma_start(out=idxLt[:], in_=d["idxL"][:])
            idxHt = cst.tile([128, NG * GS_H // 16], i16, name="idxHt")
            nc.sync.dma_start(out=idxHt[:], in_=d["idxH"][:])

            ones1 = cst.tile([1, 128], bf16, name="ones1")
            nc.gpsimd.memset(ones1[:], 1.0)

            for layer in range(2):
                # ---- kv GEMM own nodes -> kv_own ----
                with nc.named_scope(f"kv{layer}"):
                    for t in range(WIN):
                        csl = slice(t * 128, (t + 1) * 128)
                        pkv = ps.tile([128, KVROW], f32, tag="kve", bufs=2)
                        if layer == 0:
                            xts = sb.tile([6, 128], bf16, tag="xts", bufs=3)
                            nc.sync.dma_start(out=xts[:], in_=d["x6T"][:, csl])
                            nc.tensor.matmul(pkv[:], xts[:], C["kvslab0"][:],
                                             start=True, stop=True)
                        else:
                            hta = sb.tile([128, 128], bf16, tag="hta", bufs=3)
                            nc.sync.dma_start(out=hta[:], in_=hT[1][0:128, csl])
                            htb = sb.tile([32, 128], bf16, tag="htb", bufs=3)
                            nc.sync.dma_start(out=htb[:], in_=hT[1][128:160, csl])
                            nc.tensor.matmul(pkv[:], hta[:], C["kvslab1a"][:], start=True, stop=False)
                            nc.tensor.matmul(pkv[:], htb[:], C["kvslab1b"][:], start=False, stop=False)
                            nc.tensor.matmul(pkv[:], ones1[:, :128], C["kvslab1c"][:], start=False, stop=True)
                        kvsb = sb.tile([128, KVROW], bf16, tag="kvsb")
                        nc.scalar.activation(out=kvsb[:], in_=pkv[:], func=AF.Copy)
                        nc.sync.dma_start(out=kv_own[layer][csl, :], in_=kvsb[:])
                with nc.named_scope(f"ag{layer}"):
                    nc.gpsimd.collective_compute(
                        "AllGather", mybir.AluOpType.bypass, replica_groups=rg,
                        ins=[kv_own[layer][:]], outs=[kv_full[layer][:]])

                # ---- edge phase ----
                with nc.named_scope(f"edge{layer}"):
                    cur = {"L": -1, "H": -1}
                    cur_tile = {"L": None, "H": None}

                    def _gather(region, gt):
                        if cur[region] == gt:
                            return cur_tile[region]
                        idxt, base, gsz, reg = (
                            (idxLt, 0, GS_L, regGS_L) if region == "L"
                            else (idxHt, SPLIT, GS_H, regGS_H)
                        )
                        gtile = gath.tile([128, gsz // 128, KVROW], bf16, tag="g" + region)
                        nc.gpsimd.dma_gather(
                            gtile[:],
                            kv_full[layer][base:base + SPLIT, :],
                            idxt[:, gt * (gsz // 16):(gt + 1) * (gsz // 16)],
                            num_idxs=gsz, num_idxs_reg=reg, elem_size=KVROW)
                        cur[region] = gt
                        cur_tile[region] = gtile
                        return gtile

                    for w in range(WIN):
                        wsl = slice(w * 128, (w + 1) * 128)
                        # window node GEMM -> q|C|r|-rQ
                        psq = ps.tile([128, QCW], f32, tag="pq", bufs=3)
                        if layer == 0:
                            xts = sb.tile([6, 128], bf16, tag="xts", bufs=3)
                            nc.sync.dma_start(out=xts[:], in_=d["x6T"][:, wsl])
                            nc.tensor.matmul(psq[:], xts[:], C["qcslab0"][:],
                                             start=True, stop=True)
                        else:
                            hta = sb.tile([128, 128], bf16, tag="hta", bufs=3)
                            nc.sync.dma_start(out=hta[:], in_=hT[1][0:128, wsl])
                            htb = sb.tile([32, 128], bf16, tag="htb", bufs=3)
                            nc.sync.dma_start(out=htb[:], in_=hT[1][128:160, wsl])
                            nc.tensor.matmul(psq[:], hta[:], C["qcslab1a"][:], start=True, stop=False)
                            nc.tensor.matmul(psq[:], htb[:], C["qcslab1b"][:], start=False, stop=False)
                            nc.tensor.matmul(psq[:], ones1[:, :128], C["qcslab1c"][:], start=False, stop=True)
                        qc = sb.tile([128, QCW], bf16, tag="qc", bufs=2)
                        nc.scalar.activation(out=qc[:], in_=psq[:], func=AF.Copy)

                        stqt = sb.tile([128, NCH * 128], bf16, tag="stqt", bufs=3)
                        nc.sync.dma_start(out=stqt[:], in_=d["stq"][w])
                        stst = sb.tile([128, NCH * 128], bf16, tag="stst", bufs=3)
                        nc.sync.dma_start(out=stst[:], in_=d["sts"][w])
                        eact = sb.tile([128, NCH, 8], bf16, tag="eact", bufs=3)
                        nc.sync.dma_start(out=eact[:], in_=d["eaC"][w])

                        gl = _gather("L", w)
                        gh = _gather("H", w)
                        halfL = 0
                        halfH = 0

                        # per-chunk qC one-hot gather matmuls (2 per PSUM bank)
                        qcg = sb.tile([128, NCH, QCROW], bf16, tag="qcg", bufs=2)
                        for pj in range((NCH + 1) // 2):
                            jn = min(2, NCH - pj * 2)
                            pq = ps.tile([128, 2, QCROW], f32, tag="pq", bufs=3)
                            for s in range(jn):
                                j = pj * 2 + s
                                nc.tensor.matmul(pq[:, s, :],
                                                 stqt[:, j * 128:(j + 1) * 128],
                                                 qc[:, 0:QCROW],
                                                 start=True, stop=True,
                                                 skip_group_check=True)
                            nc.scalar.activation(out=qcg[:, pj * 2:pj * 2 + jn, :],
                                                 in_=pq[:, 0:jn, :], func=AF.Copy)

                        # batched DVE per L/H group
                        stage = sb.tile([128, NCH * 4, 48], bf16, tag="stage", bufs=2)
                        al = sb.tile([128, NCH * 4], f32, tag="al", bufs=2)
                        wt = sb.tile([128, NCH, 192], bf16, tag="wt", bufs=2)
                        exs = sb.tile([128, NCH * 4], bf16, tag="exs", bufs=2)
                        for (g0, cnt, gt, half) in ((0, C_L, gl, halfL),
                                                    (C_L, C_H, gh, halfH)):
                            kvg = gt[:, half:half + cnt, :]
                            qs = qcg[:, g0:g0 + cnt, :]
                            h4 = slice(g0 * 4, (g0 + cnt) * 4)
                            # q*k -> stage[.., 0:40]
                            nc.vector.tensor_tensor(
                                out=stage[:, h4, 0:40].rearrange("p (j h) dd -> p j h dd", h=4),
                                in0=qs[:, :, 0:160].rearrange("p j (h dd) -> p j h dd", h=4),
                                in1=kvg[:, :, 0:160].rearrange("p j (h dd) -> p j h dd", h=4),
                                op=mybir.AluOpType.mult)
                            # ea*C -> stage[.., 40:44]
                            nc.vector.tensor_tensor(
                                out=stage[:, h4, 40:44].rearrange("p (j h) c -> p j h c", h=4),
                                in0=qs[:, :, 160:176].rearrange("p j (h c) -> p j h c", h=4),
                                in1=eact[:, g0:g0 + cnt, 0:4]
                                    .rearrange("p j (o c) -> p j o c", o=1)
                                    .to_broadcast([128, cnt, 4, 4]),
                                op=mybir.AluOpType.mult)
                            # alpha = reduce
                            nc.vector.tensor_reduce(
                                out=al[:, h4], in_=stage[:, h4, 0:44],
                                axis=mybir.AxisListType.X, op=mybir.AluOpType.add)
                        # ex = exp(alpha/sqrt(D))
                        nc.scalar.activation(out=exs[:], in_=al[:], func=AF.Exp,
                                             scale=INVSQD)
                        for (g0, cnt, gt, half) in ((0, C_L, gl, halfL),
                                                    (C_L, C_H, gh, halfH)):
                            kvg = gt[:, half:half + cnt, :]
                            h4 = slice(g0 * 4, (g0 + cnt) * 4)
                            # wt v-block = v_g * ex
                            nc.vector.tensor_tensor(
                                out=wt[:, g0:g0 + cnt, 0:160].rearrange("p j (h dd) -> p j h dd", h=4),
                                in0=kvg[:, :, 160:320].rearrange("p j (h dd) -> p j h dd", h=4),
                                in1=exs[:, h4].rearrange("p (j h o) -> p j h o", h=4, o=1)
                                    .to_broadcast([128, cnt, 4, 40]),
                                op=mybir.AluOpType.mult)
                            # wt S-block = ea * ex
                            nc.vector.tensor_tensor(
                                out=wt[:, g0:g0 + cnt, 160:176].rearrange("p j (h c) -> p j h c", h=4),
                                in0=eact[:, g0:g0 + cnt, 0:4]
                                    .rearrange("p j (o c) -> p j o c", o=1)
                                    .to_broadcast([128, cnt, 4, 4]),
                                in1=exs[:, h4].rearrange("p (j h o) -> p j h o", h=4, o=1)
                                    .to_broadcast([128, cnt, 4, 4]),
                                op=mybir.AluOpType.mult)
                            # wt denom block = ex
                            nc.vector.tensor_copy(
                                out=wt[:, g0:g0 + cnt, 176:180],
                                in_=exs[:, h4].rearrange("p (j h) -> p j h", h=4))

                        # scatter: acc[nodes, (h,48)] += st^T @ wt
                        pacc = ps.tile([128, 192], f32, tag="acc", bufs=2)
                        for j in range(NCH):
                            nc.tensor.matmul(pacc[:],
                                             stst[:, j * 128:(j + 1) * 128],
                                             wt[:, j, :],
                                             start=(j == 0), stop=(j == NCH - 1),
                                             skip_group_check=True)

                        # ---- window post ----
                        accsb = sb.tile([128, 192], bf16, tag="accsb")
                        nc.scalar.activation(out=accsb[:], in_=pacc[:], func=AF.Copy)
                        # S correction: transpose accS [128, (h,c<4)] -> [16,128]
                        pst = ps.tile([16, 128], bf16, tag="tp", bufs=1)
                        nc.tensor.transpose(pst[:], accsb[:, 160:176], C["ident"][:])
                        tS = sb.tile([16, 128], bf16, tag="tS")
                        nc.vector.tensor_copy(out=tS[:], in_=pst[:])
                        pcorr = ps.tile([128, HID], f32, tag="tp", bufs=1)
                        nc.tensor.matmul(pcorr[:], tS[:], C[f"wep{layer}"][:],
                                         start=True, stop=True)
                        # outn = (accv + corr) * 1/denom
                        outn0 = sb.tile([128, HID], bf16, tag="outn0")
                        nc.vector.tensor_tensor(
                            out=outn0[:], in0=accsb[:, 0:160], in1=pcorr[:],
                            op=mybir.AluOpType.add)
                        dmax = sb.tile([128, 4], f32, tag="dmax")
                        nc.vector.tensor_scalar_max(dmax[:], accsb[:, 176:180], 1e-30)
                        denr = sb.tile([128, 4], f32, tag="denr")
                        nc.vector.reciprocal(out=denr[:], in_=dmax[:])
                        outn = sb.tile([128, HID], bf16, tag="outn")
                        nc.vector.tensor_tensor(
                            out=outn[:].rearrange("p (h dd) -> p h dd", h=4),
                            in0=outn0[:].rearrange("p (h dd) -> p h dd", h=4),
                            in1=denr[:].rearrange("p (h o) -> p h o", o=1)
                                .to_broadcast([128, 4, 40]),
                            op=mybir.AluOpType.mult)
                        # beta gate
                        scr = sb.tile([128, HID], bf16, tag="scr")
                        nc.vector.tensor_tensor(out=scr[:], in0=outn[:],
                                                in1=C[f"prep{layer}"][:],
                                                op=mybir.AluOpType.mult)
                        outP = sb.tile([128, 1], f32, tag="outP")
                        nc.vector.tensor_reduce(
                            out=outP[:], in_=scr[:].rearrange("p (a b) -> p a b", a=1),
                            axis=mybir.AxisListType.XY, op=mybir.AluOpType.add)
                        exb = sb.tile([128, 1], f32, tag="exb")
                        nc.scalar.activation(out=exb[:], in_=outP[:], func=AF.Exp,
                                             scale=-1.0, bias=qc[:, 352:353])
                        betad = sb.tile([128, 1], f32, tag="betad")
                        nc.vector.tensor_scalar_add(betad[:], exb[:], 1.0)
                        beta = sb.tile([128, 1], f32, tag="beta")
                        nc.vector.reciprocal(out=beta[:], in_=betad[:])
                        dvec = sb.tile([128, HID], bf16, tag="dvec")
                        nc.vector.tensor_sub(dvec[:], qc[:, QCROW:QCROW + 160], outn[:])
                        hp = sb.tile([128, HID], bf16, tag="hp")
                        nc.vector.scalar_tensor_tensor(
                            out=hp[:], in0=dvec[:], scalar=beta[:, 0:1], in1=outn[:],
                            op0=mybir.AluOpType.mult, op1=mybir.AluOpType.add)
                        nc.sync.dma_start(out=h_nm[layer + 1][wsl, :], in_=hp[:])
                        ptr1 = ps.tile([128, 128], bf16, tag="tp", bufs=1)
                        nc.tensor.transpose(ptr1[:], hp[:, 0:128], C["ident"][:])
                        t1 = sb.tile([128, 128], bf16, tag="t1")
                        nc.vector.tensor_copy(out=t1[:], in_=ptr1[:])
                        nc.sync.dma_start(out=hT[layer + 1][0:128, wsl], in_=t1[:])
                        ptr2 = ps.tile([32, 128], bf16, tag="tp", bufs=1)
                        nc.tensor.transpose(ptr2[:], hp[:, 128:160], C["ident"][:])
                        t2 = sb.tile([32, 128], bf16, tag="t2")
                        nc.vector.tensor_copy(out=t2[:], in_=ptr2[:])
                        nc.sync.dma_start(out=hT[layer + 1][128:160, wsl], in_=t2[:])

            # ==== final phase: gate + graph pooling + head MLP ====
            with nc.named_scope("final"):
                pgr = ps.tile([32, 321], f32, tag="acc", bufs=2)
                for w in range(WIN):
                    wsl = slice(w * 128, (w + 1) * 128)
                    h1w = sb.tile([128, HID], bf16, tag="h1w")
                    nc.sync.dma_start(out=h1w[:], in_=h_nm[1][wsl, :])
                    h2w = sb.tile([128, HID], bf16, tag="h2w")
                    nc.sync.dma_start(out=h2w[:], in_=h_nm[2][wsl, :])
                    sgt = sb.tile([128, 32], bf16, tag="sgt", bufs=3)
                    nc.sync.dma_start(out=sgt[:], in_=d["sgw"][w])
                    pg = ps.tile([128, HID], f32, tag="kve", bufs=2)
                    first = True
                    for (src_hT, wkey) in ((hT[1], "wg1h1"), (hT[2], "wg1h2")):
                        g_a = sb.tile([128, 128], bf16, tag="hta", bufs=3)
                        nc.sync.dma_start(out=g_a[:], in_=src_hT[0:128, wsl])
                        g_b = sb.tile([32, 128], bf16, tag="htb", bufs=3)
                        nc.sync.dma_start(out=g_b[:], in_=src_hT[128:160, wsl])
                        nc.tensor.matmul(pg[:], g_a[:], C[wkey + "a"][:], start=first, stop=False)
                        first = False
                        nc.tensor.matmul(pg[:], g_b[:], C[wkey + "b"][:], start=False, stop=False)
                    nc.tensor.matmul(pg[:], ones1[:, :128], C["wg1bias"][:], start=False, stop=True)
                    grelu = sb.tile([128, HID], bf16, tag="grelu")
                    nc.scalar.activation(out=grelu[:], in_=pg[:], func=AF.Relu)
                    scr2 = sb.tile([128, HID], bf16, tag="scr")
                    gatec = sb.tile([128, 1], f32, tag="gatec")
                    nc.vector.tensor_tensor(out=scr2[:], in0=grelu[:],
                                            in1=C["wg2rep"][:], op=mybir.AluOpType.mult)
                    nc.vector.tensor_reduce(
                        out=gatec[:], in_=scr2[:].rearrange("p (a b) -> p a b", a=1),
                        axis=mybir.AxisListType.XY, op=mybir.AluOpType.add)
                    ge = sb.tile([128, 1], f32, tag="ge")
                    nc.scalar.activation(out=ge[:], in_=gatec[:], func=AF.Exp,
                                         bias=C["bg2rep"][:, 0:1])
                    wg = sb.tile([128, 321], bf16, tag="wg")
                    nc.vector.tensor_scalar_mul(wg[:, 0:HID], h1w[:], ge[:, 0:1])
                    nc.vector.tensor_scalar_mul(wg[:, HID:2 * HID], h2w[:], ge[:, 0:1])
                    nc.vector.tensor_copy(out=wg[:, 320:321], in_=ge[:])
                    nc.tensor.matmul(pgr[:], sgt[:], wg[:], start=(w == 0),
                                     stop=(w == WIN - 1), skip_group_check=True)
                pg_sb = sb.tile([32, 321], f32, tag="pg_sb")
                nc.vector.tensor_copy(out=pg_sb[:], in_=pgr[:])
                nc.sync.dma_start(out=pool_in[:], in_=pg_sb[:])
                nc.gpsimd.collective_compute(
                    "AllReduce", mybir.AluOpType.add, replica_groups=rg,
                    ins=[pool_in[:]], outs=[pool_out[:]])
                psb = sb.tile([32, 321], f32, tag="psb")
                nc.sync.dma_start(out=psb[:], in_=pool_out[:])
                gden = sb.tile([32, 1], f32, tag="gden")
                nc.vector.tensor_scalar_max(gden[:], psb[:, 320:321], 1e-30)
                gdr = sb.tile([32, 1], f32, tag="gdr")
                nc.vector.reciprocal(out=gdr[:], in_=gden[:])
                pl = sb.tile([32, 320], bf16, tag="pl")
                nc.vector.tensor_scalar_mul(pl[:], psb[:, 0:320], gdr[:, 0:1])

                def _headmm(vin, wa, wb, wc, wd, nout, tagp):
                    pouts = ps.tile([32, nout], f32, tag=tagp, bufs=(3 if tagp == "pq" else 2))
                    for si, (c0, m) in enumerate(((0, 128), (128, 128), (256, 64))):
                        ptt = ps.tile([m, 32], bf16, tag="tp", bufs=1)
                        nc.tensor.transpose(ptt[:], vin[:, c0:c0 + m], C["ident"][0:32, 0:32])
                        tsb = sb.tile([m, 32], bf16, tag="tsb")
                        nc.vector.tensor_copy(out=tsb[:], in_=ptt[:])
                        nc.tensor.matmul(pouts[:], tsb[:], (wa, wb, wc)[si][:m, :],
                                         start=(si == 0), stop=False, skip_group_check=True)
                    nc.tensor.matmul(pouts[:], ones1[:, :32], wd[:],
                                     start=False, stop=True, skip_group_check=True)
                    return pouts

                ph1 = _headmm(pl, C["wh1a"], C["wh1b"], C["wh1c"], C["wh1d"], 320, "pq")
                vrel = sb.tile([32, 320], bf16, tag="vrel")
                nc.scalar.activation(out=vrel[:], in_=ph1[:], func=AF.Relu)
                ph2 = _headmm(vrel, C["wh2a"], C["wh2b"], C["wh2c"], C["wh2d"], 6, "kve")
                osb = sb.tile([32, 6], f32, tag="osb")
                nc.vector.tensor_copy(out=osb[:], in_=ph2[:])
                nc.sync.dma_start(out=out_d[:], in_=osb[:])
                dbgt = sb.tile([128, KVROW], f32, tag="dbgt")
                nc.gpsimd.memset(dbgt[:], 0.0)
                nc.sync.dma_start(out=dbg_d[:], in_=dbgt[:])

    nc.compile()
    return nc


_CACHE = {}
_LAST_RES = None


def kernel(**inputs):
    inputs = {k: np.asarray(v) for k, v in inputs.items()}
    per_core, C_L, C_H = _preprocess(
        inputs["x"], inputs["edge_index"], inputs["edge_attr"], inputs["batch"])
    w = _weights(inputs)
    key = (C_L, C_H)
    if key not in _CACHE:
        _CACHE[key] = _build(C_L, C_H)
    nc = _CACHE[key]
    in_maps = []
    for r in range(NCORES):
        m = dict(w)
        m.update(per_core[r])
        in_maps.append(m)
    import os
    trace = bool(os.environ.get("KERNEL_TRACE"))
    if trace:
        try:
            import axon_prof
            axon_prof.install()
        except Exception:
            trace = False
    res = run_bass_kernel_spmd(nc, in_maps, core_ids=list(range(NCORES)), trace=trace)
    if trace and res.exec_time_ns is not None:
        print(f"HW exec time: {res.exec_time_ns} ns")
        if res.per_core_scope_times:
            for scope, cores in sorted(res.per_core_scope_times.items()):
                print(f"  scope {scope}: {cores}")
    global _LAST_RES
    _LAST_RES = res
    out = res.results[0]["out"]
    return out.reshape(G, 2, 3).astype(np.float32)
